# revision 44
# baseline (speedup 1.0000x reference)
"""KANLinear (no residual) Trainium2 kernel.

out[b,o] = sum_{i,g} B_g(x[b,i]) * W[o,i,g] where B_g are cubic B-spline
bases on a uniform grid (G=5, k=3, range [-1,1] -> 8 bases, knots
t_j = 0.4*j - 2.2).

Closed form used on-device: with u = 2.5*x + 5.5 - g and the fold
z = min(u, 4-u) (= 2 - |u-2|),

    B_g(x) = relu(z*c1)^3 - relu((z-1)*c2)^3
    c1 = 6^(-1/3),  c2 = (2/3)^(1/3)
    relu(z)   = relu(min(u, 4-u))      (min-of-two-affines, no abs needed)
    relu(z-1) = relu(min(u-1, 3-u))

which is exact for the cardinal cubic B-spline everywhere. Two custom DVE
ops per basis plane:
    HINGE1    (7 stages): h1 = cube(relu(min(x*s0 - s1, imm2 - x*s0)))
    HINGE2SUB (8 stages): B  = h1 - cube(relu(min(x*s0 - s1, imm2 - x*s0)))
The second op folds the h1-h2 subtraction, so no separate tensor_sub pass
is needed, and it writes the basis plane directly in bf16 for the matmul.

The big matmul runs in bf16 (1 PE cycle/row at 2.4 GHz; fp32r on HW is a
2-pass mode at ~2x the time). x, bases, weights and the DRAM output are
all bf16; PSUM accumulates in fp32. Measured relative error 4.4e-3 vs
the 2e-2 gate.

Sharding: data-parallel over tokens (4096 -> 512 per core on 8 cores),
spline_weight replicated (bf16, 16 MB/core streamed); no collectives,
host concatenates the shards.

Schedule: per (i-tile t, basis g): 2 DVE ops (~1.4us) feed 8 matmuls
(~1.73us); weight DMAs alternate over the Sync/GpSimd DGE rings and x
rides the Scalar ring so the PE is never DMA-starved. A short junk-matmul
warmup keeps the PE p-state hot until the first bases land; the first
slice is split in halves across rings/ops so the stream locks ~11us
after launch. PSUM is evicted via Scalar/Vector copies (bf16) with the
final chunk split across both engines and two DMA rings to shorten the
drain tail. Measured ~131-134us/core on trn2 (tensor-engine floor for
this shape is 512 matmuls x 216ns = 110.6us + ~13.5us fixed NEFF
preamble/teardown + data-arrival latency).
"""

import numpy as np

N_CORES = 8
B_TOT = 4096
B_SHARD = B_TOT // N_CORES  # 512
IN_F = 1024
OUT_F = 1024
G = 8  # GRID_SIZE + SPLINE_ORDER
I_TILES = IN_F // 128  # 8
M_TILES = B_SHARD // 128  # 4
N_CHUNKS = OUT_F // 512  # 2

_C1 = float(6.0 ** (-1.0 / 3.0))
_C2 = float((2.0 / 3.0) ** (1.0 / 3.0))

_CACHE = {}


def _dve_ops():
    """Register (once) and return the two custom DVE ops."""
    if "ops" in _CACHE:
        return _CACHE["ops"]

    from concourse import dve_ops
    from concourse.dve_ops import DveOp, has_src1
    from concourse.dve_spec import C0, C1, C2, Spec, Src0, Src1, lower, minn, relu, sq
    from concourse.dve_uop import DveOpSpec

    def cube(t):
        return sq(t) * t

    def _ref1(in0, in1, s0, s1, imm2):
        m = in0.astype(np.float32) * np.float32(s0)
        w = np.minimum(m - np.float32(s1), np.float32(imm2) - m)
        t = np.maximum(w, np.float32(0.0))
        return (t * t * t).astype(np.float32)

    def _ref2(in0, in1, s0, s1, imm2):
        m = in1.astype(np.float32) * np.float32(s0)
        w = np.minimum(m - np.float32(s1), np.float32(imm2) - m)
        t = np.maximum(w, np.float32(0.0))
        return (in0.astype(np.float32) - t * t * t).astype(np.float32)

    m1 = Src0 * C0
    spec1 = Spec(body=cube(relu(minn(m1 - C1, C2 - m1))), reference=_ref1)
    m2 = Src1 * C0
    spec2 = Spec(body=Src0 - cube(relu(minn(m2 - C1, C2 - m2))), reference=_ref2)

    ops = []
    for name, spec in (("BSPL_HINGE1", spec1), ("BSPL_HINGE2SUB", spec2)):
        if name not in dve_ops._SUB_OPCODE_FOR_NAME:
            row = dve_ops._CUSTOM_DVE_ROW_BASE + len(dve_ops.OPS)
            assert row < 0x20
            shas = {}
            for ver in ("v3", "v4"):
                try:
                    tmp = DveOpSpec(
                        name=name,
                        opcode=row,
                        uops=lower(spec, ver=ver),
                        rd1_en=has_src1(spec),
                    )
                    shas[ver] = tmp.sha(ver)
                except Exception:
                    pass
            op = DveOp(name, spec, subdim=False, uops_sha=shas)
            dve_ops.OPS.append(op)
            dve_ops._SUB_OPCODE_FOR_NAME[name] = row
            dve_ops.CUSTOM_DVE_SPECS[name] = spec
        else:
            op = next(o for o in dve_ops.OPS if o.name == name)
        ops.append(op)

    _CACHE["ops"] = tuple(ops)
    return _CACHE["ops"]


def _build_nc():
    """Build the per-core Bass program (SPMD: identical on all 8 cores)."""
    if "nc" in _CACHE:
        return _CACHE["nc"]

    from concourse import bacc
    import concourse.mybir as mybir
    import concourse.tile as tile

    op1, op2 = _dve_ops()

    f32 = mybir.dt.float32
    bf16 = mybir.dt.bfloat16

    nc = bacc.Bacc(None, target_bir_lowering=False)

    x_t = nc.declare_dram_parameter("x_t", [IN_F, B_SHARD], bf16, isOutput=False)
    w = nc.declare_dram_parameter("w", [G * IN_F, OUT_F], bf16, isOutput=False)
    out = nc.declare_dram_parameter("out", [B_SHARD, OUT_F], bf16, isOutput=True)

    with tile.TileContext(nc) as tc:
        with (
            tc.tile_pool(name="xp", bufs=4) as xp,
            tc.tile_pool(name="hp", bufs=3) as hp,
            tc.tile_pool(name="bp", bufs=14) as bp,
            tc.tile_pool(name="wp", bufs=12) as wp,
            tc.tile_pool(name="op", bufs=8) as op_,
            tc.tile_pool(name="scr", bufs=1) as scrp,
            tc.tile_pool(name="ps", bufs=1, space="PSUM") as ps,
        ):
            psum = [
                [
                    ps.tile([128, 512], f32, tag=f"ps_{m}_{n}", name=f"ps_{m}_{n}")
                    for n in range(N_CHUNKS)
                ]
                for m in range(M_TILES)
            ]

            # PE warmup: ~3us of junk matmuls on a memset scratch tile so the
            # p-state governor reaches full clock before the real stream.
            scr = scrp.tile([128, 640], bf16, tag="scr")
            nc.vector.memset(scr[:, :], 0.0)
            N_WARM = 5
            for i in range(N_WARM):
                nc.tensor.matmul(
                    psum[0][0][:, :],
                    scr[:, 0:128],
                    scr[:, 128:640],
                    start=i == 0,
                    stop=i == N_WARM - 1,
                )

            # wt DMAs alternate between the Sync and GpSimd DGE rings; xt
            # rides the Scalar ring (t=0 split across Sync+Scalar so the
            # first bases start earlier), so neither steals weight bandwidth.
            def emit_plane(xt, g, lo, hi):
                h1 = hp.tile([128, hi - lo], f32, tag="h1")
                nc.vector._custom_dve(
                    op1,
                    out=h1[:, :],
                    in0=xt[:, lo:hi],
                    s0=2.5 * _C1,
                    s1=(g - 5.5) * _C1,
                    imm2=(g - 1.5) * _C1,
                )
                bb = bp.tile([128, hi - lo], bf16, tag="bb")
                nc.vector._custom_dve(
                    op2,
                    out=bb[:, :],
                    in0=h1[:, :],
                    in1=xt[:, lo:hi],
                    s0=2.5 * _C2,
                    s1=(g - 4.5) * _C2,
                    imm2=(g - 2.5) * _C2,
                )
                return bb

            def emit_wt(t, g):
                wt = wp.tile([128, OUT_F], bf16, tag="wt")
                r0 = g * IN_F + t * 128
                weng = nc.sync if g % 2 == 0 else nc.gpsimd
                weng.dma_start(out=wt[:, :], in_=w[r0 : r0 + 128, :])
                return wt

            def emit_mms(bb, off, wt, first, last):
                for m in range(M_TILES):
                    lhsT = bb[:, off + m * 128 : off + (m + 1) * 128]
                    for n in range(N_CHUNKS):
                        nc.tensor.matmul(
                            psum[m][n][:, :],
                            lhsT,
                            wt[:, n * 512 : (n + 1) * 512],
                            start=first,
                            stop=last,
                        )

            # tiles 0 and 1: per-tile planes (startup path: first slice split
            # into halves across rings/ops so the stream locks early)
            for t in (0, 1):
                xt = xp.tile([128, B_SHARD], bf16, tag="xt")
                src = x_t[t * 128 : (t + 1) * 128, :]
                if t == 0:
                    nc.sync.dma_start(out=xt[:, 0:256], in_=src[:, 0:256])
                    nc.scalar.dma_start(out=xt[:, 256:512], in_=src[:, 256:512])
                else:
                    nc.scalar.dma_start(out=xt[:, :], in_=src)

                for g in range(G):
                    if t == 0 and g == 0:
                        wt = wp.tile([128, OUT_F], bf16, tag="wt")
                        r0 = g * IN_F
                        nc.gpsimd.dma_start(out=wt[:, 0:512], in_=w[r0 : r0 + 128, 0:512])
                        nc.scalar.dma_start(
                            out=wt[:, 512:1024], in_=w[r0 : r0 + 128, 512:1024]
                        )
                        bba = emit_plane(xt, g, 0, 256)
                        bbb = emit_plane(xt, g, 256, 512)
                        for n in range(N_CHUNKS):
                            for m in range(M_TILES):
                                bb, lo = (bba, 0) if m < 2 else (bbb, 256)
                                nc.tensor.matmul(
                                    psum[m][n][:, :],
                                    bb[:, m * 128 - lo : (m + 1) * 128 - lo],
                                    wt[:, n * 512 : (n + 1) * 512],
                                    start=True,
                                    stop=False,
                                )
                        continue
                    wt = emit_wt(t, g)
                    bb = emit_plane(xt, g, 0, 512)
                    emit_mms(bb, 0, wt, False, False)

            # tiles 2..7 in pairs: one 1024-wide DVE op pair produces the
            # basis plane for BOTH i-tiles (same g => same constants), halving
            # DVE op count/overhead and tile-boundary events
            for t0 in (2, 4, 6):
                t1 = t0 + 1
                xt2 = xp.tile([128, 2 * B_SHARD], bf16, tag="xt2")
                nc.scalar.dma_start(
                    out=xt2[:, 0:512], in_=x_t[t0 * 128 : (t0 + 1) * 128, :]
                )
                nc.scalar.dma_start(
                    out=xt2[:, 512:1024], in_=x_t[t1 * 128 : (t1 + 1) * 128, :]
                )
                pair_bbs = []
                for g in range(G):
                    wt = emit_wt(t0, g)
                    bb = emit_plane(xt2, g, 0, 1024)
                    pair_bbs.append(bb)
                    emit_mms(bb, 0, wt, False, False)
                for g in range(G):
                    wt = emit_wt(t1, g)
                    last = t1 == I_TILES - 1 and g == G - 1
                    emit_mms(pair_bbs[g], 512, wt, False, last)

            dengs = [nc.sync, nc.gpsimd, nc.sync, nc.gpsimd, nc.sync, nc.gpsimd, nc.scalar]
            for m in range(M_TILES):
                for n in range(N_CHUNKS):
                    k = m * N_CHUNKS + n
                    ot = op_.tile([128, 512], bf16, tag="ot")
                    if k < 7:
                        nc.scalar.copy(out=ot[:, 0:256], in_=psum[m][n][:, 0:256])
                        nc.vector.tensor_copy(
                            out=ot[:, 256:512], in_=psum[m][n][:, 256:512]
                        )
                        dengs[k].dma_start(
                            out=out[m * 128 : (m + 1) * 128, n * 512 : (n + 1) * 512],
                            in_=ot[:, :],
                        )
                    else:
                        # final chunk: split the copy across both PSUM-capable
                        # engines and its DMA across two idle rings to shorten
                        # the drain tail
                        nc.scalar.copy(out=ot[:, 0:256], in_=psum[m][n][:, 0:256])
                        nc.vector.tensor_copy(
                            out=ot[:, 256:512], in_=psum[m][n][:, 256:512]
                        )
                        nc.sync.dma_start(
                            out=out[m * 128 : (m + 1) * 128, n * 512 : n * 512 + 256],
                            in_=ot[:, 0:256],
                        )
                        nc.gpsimd.dma_start(
                            out=out[m * 128 : (m + 1) * 128, n * 512 + 256 : (n + 1) * 512],
                            in_=ot[:, 256:512],
                        )

    nc.finalize()
    _CACHE["nc"] = nc
    return nc


def _in_maps(x, w2):
    import ml_dtypes

    maps = []
    for c in range(N_CORES):
        xs = x[c * B_SHARD : (c + 1) * B_SHARD, :]
        maps.append({"x_t": np.ascontiguousarray(xs.T.astype(ml_dtypes.bfloat16)), "w": w2})
    return maps


def kernel(x, spline_weight, _trace=False):
    import ml_dtypes

    x = np.ascontiguousarray(np.asarray(x, dtype=np.float32))
    W = np.asarray(spline_weight, dtype=np.float32)
    assert x.shape == (B_TOT, IN_F) and W.shape == (OUT_F, IN_F, G)

    # w2[g*IN_F + i, o] = W[o, i, g]
    w2 = np.ascontiguousarray(
        W.transpose(2, 1, 0).reshape(G * IN_F, OUT_F).astype(ml_dtypes.bfloat16)
    )

    from concourse.bass_utils import run_bass_kernel_spmd

    nc = _build_nc()
    res = run_bass_kernel_spmd(nc, _in_maps(x, w2), list(range(N_CORES)), trace=_trace)
    out = np.concatenate(
        [np.asarray(res.results[c]["out"]) for c in range(N_CORES)], axis=0
    )
    if _trace:
        _CACHE["last_result"] = res
    return out.astype(np.float32)


# revision 45
# speedup vs baseline: 1.0082x; 1.0082x over previous
"""KANLinear (no residual) Trainium2 kernel.

out[b,o] = sum_{i,g} B_g(x[b,i]) * W[o,i,g] where B_g are cubic B-spline
bases on a uniform grid (G=5, k=3, range [-1,1] -> 8 bases, knots
t_j = 0.4*j - 2.2).

Closed form used on-device: with u = 2.5*x + 5.5 - g and the fold
z = min(u, 4-u) (= 2 - |u-2|),

    B_g(x) = relu(z*c1)^3 - relu((z-1)*c2)^3
    c1 = 6^(-1/3),  c2 = (2/3)^(1/3)
    relu(z)   = relu(min(u, 4-u))      (min-of-two-affines, no abs needed)
    relu(z-1) = relu(min(u-1, 3-u))

which is exact for the cardinal cubic B-spline everywhere. Two custom DVE
ops per basis plane:
    HINGE1    (7 stages): h1 = cube(relu(min(x*s0 - s1, imm2 - x*s0)))
    HINGE2SUB (8 stages): B  = h1 - cube(relu(min(x*s0 - s1, imm2 - x*s0)))
The second op folds the h1-h2 subtraction, so no separate tensor_sub pass
is needed, and it writes the basis plane directly in bf16 for the matmul.

The big matmul runs in bf16 (1 PE cycle/row at 2.4 GHz; fp32r on HW is a
2-pass mode at ~2x the time). x, bases, weights and the DRAM output are
all bf16; PSUM accumulates in fp32. Measured relative error 4.4e-3 vs
the 2e-2 gate.

Sharding: data-parallel over tokens (4096 -> 512 per core on 8 cores),
spline_weight replicated (bf16, 16 MB/core streamed); no collectives,
host concatenates the shards.

Schedule: per (i-tile t, basis g): 2 DVE ops (~1.4us) feed 8 matmuls
(~1.73us); weight DMAs alternate over the Sync/GpSimd DGE rings and x
rides the Scalar ring so the PE is never DMA-starved. A short junk-matmul
warmup keeps the PE p-state hot until the first bases land; the first
slice is split in halves across rings/ops so the stream locks ~11us
after launch. PSUM is evicted via Scalar/Vector copies (bf16) with the
final chunk split across both engines and two DMA rings to shorten the
drain tail. Measured ~131-134us/core on trn2 (tensor-engine floor for
this shape is 512 matmuls x 216ns = 110.6us + ~13.5us fixed NEFF
preamble/teardown + data-arrival latency).
"""

import numpy as np

N_CORES = 8
B_TOT = 4096
B_SHARD = B_TOT // N_CORES  # 512
IN_F = 1024
OUT_F = 1024
G = 8  # GRID_SIZE + SPLINE_ORDER
I_TILES = IN_F // 128  # 8
M_TILES = B_SHARD // 128  # 4
N_CHUNKS = OUT_F // 512  # 2

_C1 = float(6.0 ** (-1.0 / 3.0))
_C2 = float((2.0 / 3.0) ** (1.0 / 3.0))

_CACHE = {}


def _dve_ops():
    """Register (once) and return the two custom DVE ops."""
    if "ops" in _CACHE:
        return _CACHE["ops"]

    from concourse import dve_ops
    from concourse.dve_ops import DveOp, has_src1
    from concourse.dve_spec import C0, C1, C2, Spec, Src0, Src1, lower, minn, relu, sq
    from concourse.dve_uop import DveOpSpec

    def cube(t):
        return sq(t) * t

    def _ref1(in0, in1, s0, s1, imm2):
        m = in0.astype(np.float32) * np.float32(s0)
        w = np.minimum(m - np.float32(s1), np.float32(imm2) - m)
        t = np.maximum(w, np.float32(0.0))
        return (t * t * t).astype(np.float32)

    def _ref2(in0, in1, s0, s1, imm2):
        m = in1.astype(np.float32) * np.float32(s0)
        w = np.minimum(m - np.float32(s1), np.float32(imm2) - m)
        t = np.maximum(w, np.float32(0.0))
        return (in0.astype(np.float32) - t * t * t).astype(np.float32)

    m1 = Src0 * C0
    spec1 = Spec(body=cube(relu(minn(m1 - C1, C2 - m1))), reference=_ref1)
    m2 = Src1 * C0
    spec2 = Spec(body=Src0 - cube(relu(minn(m2 - C1, C2 - m2))), reference=_ref2)

    ops = []
    for name, spec in (("BSPL_HINGE1", spec1), ("BSPL_HINGE2SUB", spec2)):
        if name not in dve_ops._SUB_OPCODE_FOR_NAME:
            row = dve_ops._CUSTOM_DVE_ROW_BASE + len(dve_ops.OPS)
            assert row < 0x20
            shas = {}
            for ver in ("v3", "v4"):
                try:
                    tmp = DveOpSpec(
                        name=name,
                        opcode=row,
                        uops=lower(spec, ver=ver),
                        rd1_en=has_src1(spec),
                    )
                    shas[ver] = tmp.sha(ver)
                except Exception:
                    pass
            op = DveOp(name, spec, subdim=False, uops_sha=shas)
            dve_ops.OPS.append(op)
            dve_ops._SUB_OPCODE_FOR_NAME[name] = row
            dve_ops.CUSTOM_DVE_SPECS[name] = spec
        else:
            op = next(o for o in dve_ops.OPS if o.name == name)
        ops.append(op)

    _CACHE["ops"] = tuple(ops)
    return _CACHE["ops"]


def _build_nc():
    """Build the per-core Bass program (SPMD: identical on all 8 cores)."""
    if "nc" in _CACHE:
        return _CACHE["nc"]

    from concourse import bacc
    import concourse.mybir as mybir
    import concourse.tile as tile

    op1, op2 = _dve_ops()

    f32 = mybir.dt.float32
    bf16 = mybir.dt.bfloat16

    nc = bacc.Bacc(None, target_bir_lowering=False)

    x_t = nc.declare_dram_parameter("x_t", [IN_F, B_SHARD], bf16, isOutput=False)
    w = nc.declare_dram_parameter("w", [G * IN_F, OUT_F], bf16, isOutput=False)
    out = nc.declare_dram_parameter("out", [B_SHARD, OUT_F], bf16, isOutput=True)

    with tile.TileContext(nc) as tc:
        with (
            tc.tile_pool(name="xp", bufs=4) as xp,
            tc.tile_pool(name="hp", bufs=3) as hp,
            tc.tile_pool(name="bp", bufs=14) as bp,
            tc.tile_pool(name="wp", bufs=12) as wp,
            tc.tile_pool(name="op", bufs=8) as op_,
            tc.tile_pool(name="scr", bufs=1) as scrp,
            tc.tile_pool(name="ps", bufs=1, space="PSUM") as ps,
        ):
            psum = [
                [
                    ps.tile([128, 512], f32, tag=f"ps_{m}_{n}", name=f"ps_{m}_{n}")
                    for n in range(N_CHUNKS)
                ]
                for m in range(M_TILES)
            ]

            # PE warmup: ~3us of junk matmuls on a memset scratch tile so the
            # p-state governor reaches full clock before the real stream.
            scr = scrp.tile([128, 640], bf16, tag="scr")
            nc.vector.memset(scr[:, :], 0.0)
            N_WARM = 5
            for i in range(N_WARM):
                nc.tensor.matmul(
                    psum[0][0][:, :],
                    scr[:, 0:128],
                    scr[:, 128:640],
                    start=i == 0,
                    stop=i == N_WARM - 1,
                )

            # wt DMAs alternate between the Sync and GpSimd DGE rings; xt
            # rides the Scalar ring (t=0 split across Sync+Scalar so the
            # first bases start earlier), so neither steals weight bandwidth.
            def emit_plane(xt, g, lo, hi):
                h1 = hp.tile([128, hi - lo], f32, tag="h1")
                nc.vector._custom_dve(
                    op1,
                    out=h1[:, :],
                    in0=xt[:, lo:hi],
                    s0=2.5 * _C1,
                    s1=(g - 5.5) * _C1,
                    imm2=(g - 1.5) * _C1,
                )
                bb = bp.tile([128, hi - lo], bf16, tag="bb")
                nc.vector._custom_dve(
                    op2,
                    out=bb[:, :],
                    in0=h1[:, :],
                    in1=xt[:, lo:hi],
                    s0=2.5 * _C2,
                    s1=(g - 4.5) * _C2,
                    imm2=(g - 2.5) * _C2,
                )
                return bb

            def emit_wt(t, g):
                wt = wp.tile([128, OUT_F], bf16, tag="wt")
                r0 = g * IN_F + t * 128
                weng = nc.sync if g % 2 == 0 else nc.gpsimd
                weng.dma_start(out=wt[:, :], in_=w[r0 : r0 + 128, :])
                return wt

            def emit_mms(bb, off, wt, first, last):
                for m in range(M_TILES):
                    lhsT = bb[:, off + m * 128 : off + (m + 1) * 128]
                    for n in range(N_CHUNKS):
                        nc.tensor.matmul(
                            psum[m][n][:, :],
                            lhsT,
                            wt[:, n * 512 : (n + 1) * 512],
                            start=first,
                            stop=last,
                        )

            # tiles 0 and 1: per-tile planes (startup path: first slice split
            # into halves across rings/ops so the stream locks early)
            for t in (0, 1):
                xt = xp.tile([128, B_SHARD], bf16, tag="xt")
                src = x_t[t * 128 : (t + 1) * 128, :]
                if t == 0:
                    nc.sync.dma_start(out=xt[:, 0:256], in_=src[:, 0:256])
                    nc.scalar.dma_start(out=xt[:, 256:512], in_=src[:, 256:512])
                else:
                    nc.scalar.dma_start(out=xt[:, :], in_=src)

                for g in range(G):
                    if t == 0 and g == 0:
                        wt = wp.tile([128, OUT_F], bf16, tag="wt")
                        r0 = g * IN_F
                        nc.gpsimd.dma_start(out=wt[:, 0:512], in_=w[r0 : r0 + 128, 0:512])
                        nc.scalar.dma_start(
                            out=wt[:, 512:1024], in_=w[r0 : r0 + 128, 512:1024]
                        )
                        bba = emit_plane(xt, g, 0, 256)
                        bbb = emit_plane(xt, g, 256, 512)
                        for n in range(N_CHUNKS):
                            for m in range(M_TILES):
                                bb, lo = (bba, 0) if m < 2 else (bbb, 256)
                                nc.tensor.matmul(
                                    psum[m][n][:, :],
                                    bb[:, m * 128 - lo : (m + 1) * 128 - lo],
                                    wt[:, n * 512 : (n + 1) * 512],
                                    start=True,
                                    stop=False,
                                )
                        continue
                    wt = emit_wt(t, g)
                    bb = emit_plane(xt, g, 0, 512)
                    emit_mms(bb, 0, wt, False, False)

            # tiles 2..7 in pairs: one 1024-wide DVE op pair produces the
            # basis plane for BOTH i-tiles (same g => same constants), halving
            # DVE op count/overhead and tile-boundary events
            for t0 in (2, 4, 6):
                t1 = t0 + 1
                xt2 = xp.tile([128, 2 * B_SHARD], bf16, tag="xt2")
                nc.scalar.dma_start(
                    out=xt2[:, 0:512], in_=x_t[t0 * 128 : (t0 + 1) * 128, :]
                )
                nc.scalar.dma_start(
                    out=xt2[:, 512:1024], in_=x_t[t1 * 128 : (t1 + 1) * 128, :]
                )
                pair_bbs = []
                for g in range(G):
                    wt = emit_wt(t0, g)
                    bb = emit_plane(xt2, g, 0, 1024)
                    pair_bbs.append(bb)
                    emit_mms(bb, 0, wt, False, False)
                for g in range(G):
                    wt = emit_wt(t1, g)
                    last = t1 == I_TILES - 1 and g == G - 1
                    emit_mms(pair_bbs[g], 512, wt, False, last)

            dengs = [nc.sync, nc.gpsimd, nc.sync, nc.gpsimd, nc.sync, nc.gpsimd, nc.scalar]
            for m in range(M_TILES):
                for n in range(N_CHUNKS):
                    k = m * N_CHUNKS + n
                    ot = op_.tile([128, 512], bf16, tag="ot")
                    if k < 7:
                        if n == 0:
                            nc.scalar.copy(out=ot[:, :], in_=psum[m][n][:, :])
                        else:
                            nc.vector.tensor_copy(out=ot[:, :], in_=psum[m][n][:, :])
                        dengs[k].dma_start(
                            out=out[m * 128 : (m + 1) * 128, n * 512 : (n + 1) * 512],
                            in_=ot[:, :],
                        )
                    else:
                        # final chunk: split the copy across both PSUM-capable
                        # engines and its DMA across two idle rings to shorten
                        # the drain tail
                        nc.scalar.copy(out=ot[:, 0:256], in_=psum[m][n][:, 0:256])
                        nc.vector.tensor_copy(
                            out=ot[:, 256:512], in_=psum[m][n][:, 256:512]
                        )
                        nc.sync.dma_start(
                            out=out[m * 128 : (m + 1) * 128, n * 512 : n * 512 + 256],
                            in_=ot[:, 0:256],
                        )
                        nc.gpsimd.dma_start(
                            out=out[m * 128 : (m + 1) * 128, n * 512 + 256 : (n + 1) * 512],
                            in_=ot[:, 256:512],
                        )

    nc.finalize()
    _CACHE["nc"] = nc
    return nc


def _in_maps(x, w2):
    import ml_dtypes

    maps = []
    for c in range(N_CORES):
        xs = x[c * B_SHARD : (c + 1) * B_SHARD, :]
        maps.append({"x_t": np.ascontiguousarray(xs.T.astype(ml_dtypes.bfloat16)), "w": w2})
    return maps


def kernel(x, spline_weight, _trace=False):
    import ml_dtypes

    x = np.ascontiguousarray(np.asarray(x, dtype=np.float32))
    W = np.asarray(spline_weight, dtype=np.float32)
    assert x.shape == (B_TOT, IN_F) and W.shape == (OUT_F, IN_F, G)

    # w2[g*IN_F + i, o] = W[o, i, g]
    w2 = np.ascontiguousarray(
        W.transpose(2, 1, 0).reshape(G * IN_F, OUT_F).astype(ml_dtypes.bfloat16)
    )

    from concourse.bass_utils import run_bass_kernel_spmd

    nc = _build_nc()
    res = run_bass_kernel_spmd(nc, _in_maps(x, w2), list(range(N_CORES)), trace=_trace)
    out = np.concatenate(
        [np.asarray(res.results[c]["out"]) for c in range(N_CORES)], axis=0
    )
    if _trace:
        _CACHE["last_result"] = res
    return out.astype(np.float32)
